# revision 16
# baseline (speedup 1.0000x reference)
"""Trainium2 Bass kernel for nn_CausalGatedD2Attention (v3, scan form).

Reference math (per batch): LayerNorm -> qkv proj + sigmoid-gated k,
q/k -> elu+1, quadratic causal linear attention (num = tril(q k^T) v,
den = rowsum), out = (num/den) @ w_proj + b_proj.

Sharding: 8 cores = 4 batches x 2 sequence halves (2048 tokens each).
Within a core the attention runs as a chunked linear-attention scan
(256-token chunks, running state S[dk, dv] + ksum col in SBUF, bf16).
The half-boundary state goes to the pair partner via AllGather; the
gather input (prefix6 + staged dS of chunks 6,7) is ready two chunks
before scan end so the collective overlaps the scan tail. E-dependent
terms run in a post-pass fused with the output projection; the E
contribution is gated by `flag` at the PSUM-consuming DVE op, so E
needs no zeroing pass on the first-half cores.

All matmul operands are bf16 (1 cycle/row, f32 PSUM); tolerance is
2e-2 so bf16 rounding (~4e-3) is well inside budget.
"""
import numpy as np
import ml_dtypes

import concourse.bass as bass
import concourse.tile as tile
from concourse import bacc, mybir
from concourse.bass_utils import run_bass_kernel_spmd
from concourse.masks import make_identity, make_upper_triangular

F32 = mybir.dt.float32
BF16 = mybir.dt.bfloat16
AF = mybir.ActivationFunctionType
OP = mybir.AluOpType
AX = mybir.AxisListType.X
ts = bass.ts
ds = bass.ds

BF16_NP = np.dtype(ml_dtypes.bfloat16)

P = 128
D = 1024
DK = D // P          # 8 d-chunks
LN_EPS = 1e-5
DEN_EPS = 1e-6
C = 256              # scan chunk (tokens)

B_FULL, T_FULL = 4, 4096


def _emit(tc, io, TL, use_bias):
    nc = tc.nc
    NT = TL // P         # 16 token chunks
    NCH = TL // C        # 8 scan chunks
    NSEG = TL // 512     # 4 segments for A-phase pipelining

    x, wg, wk, wq, wv, wp, flag, out = (
        io["x"], io["wg"], io["wk"], io["wq"], io["wv"], io["wp"],
        io["flag"], io["out"],
    )

    with tc.tile_pool(name="consts", bufs=1) as consts, \
         tc.tile_pool(name="dram", bufs=1, space="DRAM") as dram:
        # ---- constants ----
        ident_b = consts.tile([P, P], BF16)
        make_identity(nc, ident_b)
        tril = consts.tile([P, P], F32)   # keep s <= q (upper-tri incl diag)
        make_upper_triangular(nc, tril, val=1.0, diag=True)
        ones_f32 = consts.tile([P, 1], F32)
        nc.vector.memset(ones_f32, 1.0)
        ones_b = consts.tile([P, 1], BF16)
        nc.vector.tensor_copy(ones_b, ones_f32)
        eps_sb = consts.tile([P, 1], F32)
        nc.vector.memset(eps_sb, LN_EPS)
        flag_sb = consts.tile([P, 1], F32)
        nc.sync.dma_start(flag_sb, flag.to_broadcast([P, 1]))
        fm1 = consts.tile([P, 1], F32)    # 1 - flag
        nc.vector.tensor_scalar(fm1, flag_sb, -1.0, 1.0, op0=OP.mult, op1=OP.add)

        bias_sb = {}
        for nm in ("bq", "bk", "bg"):
            if use_bias[nm]:
                t = consts.tile([P, DK], F32, name=f"bias_{nm}")
                nc.sync.dma_start(t, io[nm].rearrange("(o p) -> p o", p=P))
                bias_sb[nm] = t
        for nm in ("bv", "bp"):
            if use_bias[nm]:
                t = consts.tile([P, D], F32, name=f"bias_{nm}")
                nc.gpsimd.dma_start(t, io[nm].partition_broadcast(P))
                bias_sb[nm] = t

        # ---- DRAM ----
        den_dram = dram.tile([TL], F32)
        cc_in = nc.dram_tensor("cc_in", [D, D + 1], BF16, kind="Internal").ap()
        cc_out = nc.dram_tensor("cc_out", [2, D, D + 1], BF16,
                                kind="Internal").ap()

        den_nof = dram.tile([TL], F32)          # den without E terms

        # ---- long-lived SBUF ----
        with tc.tile_pool(name="p_qt", bufs=1) as p_qt, \
             tc.tile_pool(name="p_big", bufs=1) as p_big:
            QT = p_qt.tile([P, DK, TL], BF16)

            with tc.tile_pool(name="p_kt", bufs=1) as p_kt, \
                 tc.tile_pool(name="p_ktok", bufs=1) as p_ktok:
                KT = p_kt.tile([P, DK, TL], BF16)
                ktok = p_ktok.tile([P, NT, D], BF16)

                # ==== Phase A: LN + transpose + gate/K/Q (seg-pipelined) ====
                # xnT shares p_big's single buffer with `num` (disjoint lives)
                if True:
                    xnT = p_big.tile([P, DK, TL], BF16, tag="big")
                    with tc.tile_pool(name="p_x", bufs=2) as p_x, \
                         tc.tile_pool(name="p_w", bufs=3) as p_w, \
                         tc.tile_pool(name="p_g", bufs=2) as p_g, \
                         tc.tile_pool(name="ps_tr", bufs=4, space="PSUM") as ps_tr, \
                         tc.tile_pool(name="ps_pj", bufs=3, space="PSUM") as ps_pj:
                        for seg in range(NSEG):
                            sl = ts(seg, 512)
                            # --- LN + transpose ---
                            for tsub in range(4):
                                t = seg * 4 + tsub
                                xt = p_x.tile([P, D], F32, tag="xt")
                                nc.sync.dma_start(xt, x[ts(t, P), :])
                                stats = p_x.tile([P, 2, 6], F32, tag="bnst")
                                for sg in range(2):
                                    nc.vector.bn_stats(out=stats[:, sg, :],
                                                       in_=xt[:, ts(sg, 512)])
                                mv = p_x.tile([P, 2], F32, tag="mv")
                                nc.vector.bn_aggr(out=mv, in_=stats)
                                nc.scalar.activation(out=mv[:, 1:2], in_=mv[:, 1:2],
                                                     func=AF.Sqrt, bias=eps_sb,
                                                     scale=1.0)
                                nc.vector.reciprocal(out=mv[:, 1:2], in_=mv[:, 1:2])
                                xn = p_x.tile([P, D], BF16, tag="xn")
                                nc.vector.tensor_scalar(xn, xt, mv[:, 0:1],
                                                        mv[:, 1:2],
                                                        op0=OP.subtract, op1=OP.mult)
                                for j in range(DK):
                                    pst = ps_tr.tile([P, P], BF16, tag="pstr")
                                    nc.tensor.transpose(pst, xn[:, ts(j, P)], ident_b)
                                    nc.any.tensor_copy(xnT[:, j, ts(t, P)], pst)

                            # --- gate, K, Q for this segment ---
                            def project(wmat, j):
                                wj = p_w.tile([P, DK, P], BF16, tag="wj")
                                nc.sync.dma_start(wj, wmat[:, ts(j, P)].rearrange(
                                    "(o p) m -> p o m", p=P))
                                ps = ps_pj.tile([P, 512], F32, tag="psproj")
                                for kc in range(DK):
                                    nc.tensor.matmul(ps, wj[:, kc], xnT[:, kc, sl],
                                                     start=(kc == 0),
                                                     stop=(kc == DK - 1))
                                return ps

                            for j in range(DK):
                                psg = project(wg, j)
                                gate = p_g.tile([P, 512], BF16, tag="gate")
                                nc.scalar.activation(
                                    out=gate, in_=psg, func=AF.Sigmoid,
                                    bias=bias_sb["bg"][:, j:j + 1] if use_bias["bg"] else 0.0)
                                psk = project(wk, j)
                                kg = p_g.tile([P, 512], F32, tag="kg")
                                nc.vector.scalar_tensor_tensor(
                                    out=kg, in0=psk,
                                    scalar=bias_sb["bk"][:, j:j + 1] if use_bias["bk"] else 0.0,
                                    in1=gate, op0=OP.add, op1=OP.mult)
                                ek = p_g.tile([P, 512], BF16, tag="ek")
                                nc.scalar.activation(out=ek, in_=kg, func=AF.Exp)
                                rk = p_g.tile([P, 512], BF16, tag="rk")
                                nc.scalar.activation(out=rk, in_=kg, func=AF.Relu)
                                nc.vector.scalar_tensor_tensor(
                                    out=KT[:, j, sl], in0=ek, scalar=1.0, in1=rk,
                                    op0=OP.min, op1=OP.add)
                            for j in range(DK):
                                psq = project(wq, j)
                                bq_ap = bias_sb["bq"][:, j:j + 1] if use_bias["bq"] else 0.0
                                eq = p_g.tile([P, 512], BF16, tag="ek")
                                nc.scalar.activation(out=eq, in_=psq, func=AF.Exp,
                                                     bias=bq_ap)
                                rq = p_g.tile([P, 512], BF16, tag="rk")
                                nc.scalar.activation(out=rq, in_=psq, func=AF.Relu,
                                                     bias=bq_ap)
                                nc.vector.scalar_tensor_tensor(
                                    out=QT[:, j, sl], in0=eq, scalar=1.0, in1=rq,
                                    op0=OP.min, op1=OP.add)
                            # --- K token-major for dS (PE transpose) ---
                            for tsub in range(4):
                                t = seg * 4 + tsub
                                for j in range(DK):
                                    pst = ps_tr.tile([P, P], BF16, tag="pstr")
                                    nc.tensor.transpose(pst, KT[:, j, ts(t, P)],
                                                        ident_b)
                                    nc.any.tensor_copy(ktok[:, t, ts(j, P)], pst)

                    # --- V projection (token-major, straight into SBUF) ---
                    p_v_cm = tc.tile_pool(name="p_v", bufs=1)
                    p_v = p_v_cm.__enter__()
                    V = p_v.tile([P, NT, D], BF16)
                    with tc.tile_pool(name="p_wv", bufs=1) as p_wv, \
                         tc.tile_pool(name="ps_v", bufs=2, space="PSUM") as ps_v:
                        wvt = p_wv.tile([P, DK, D], BF16)
                        nc.sync.dma_start(wvt, wv.rearrange("(o p) m -> p o m", p=P))
                        for t in range(NT):
                            psv = ps_v.tile([P, 2, 512], F32, tag="psv")
                            for kc in range(DK):
                                for nb in range(2):
                                    nc.tensor.matmul(
                                        psv[:, nb], xnT[:, kc, ts(t, P)],
                                        wvt[:, kc, ts(nb, 512)],
                                        start=(kc == 0), stop=(kc == DK - 1))
                            psv_flat = psv.rearrange("p a b -> p (a b)")
                            if use_bias["bv"]:
                                nc.vector.tensor_tensor(V[:, t, :], psv_flat,
                                                        bias_sb["bv"], OP.add)
                            else:
                                nc.any.tensor_copy(V[:, t, :], psv_flat)

                if "dbg_kt" in io:
                    for j in range(DK):
                        nc.sync.dma_start(io["dbg_kt"][ts(j, P), :], KT[:, j, :])
                        nc.sync.dma_start(io["dbg_qt"][ts(j, P), :], QT[:, j, :])

                # ======== scan ========
                num = p_big.tile([P, DK, TL], BF16, tag="big")  # num^T
                with tc.tile_pool(name="p_S", bufs=1) as p_S, \
                     tc.tile_pool(name="p_cc", bufs=1) as p_cc, \
                     tc.tile_pool(name="p_ssb", bufs=4) as p_ssb, \
                     tc.tile_pool(name="p_kred", bufs=4) as p_kred, \
                     tc.tile_pool(name="ps_sc", bufs=1, space="PSUM") as ps_sc, \
                     tc.tile_pool(name="ps_den", bufs=1, space="PSUM") as ps_den, \
                     tc.tile_pool(name="ps_num", bufs=1, space="PSUM") as ps_num, \
                     tc.tile_pool(name="ps_dS", bufs=2, space="PSUM") as ps_dS:
                    S = p_S.tile([P, DK, D + 1], BF16)
                    nc.vector.memset(S, 0.0)
                    # ---- stage tail state (chunks 5-7 dS, pre-scaled by
                    # 1-flag) so the AllGather can launch after chunk 4 ----
                    ccs2 = p_cc.tile([P, DK, D + 1], BF16)
                    for dkc in range(DK):
                        psS = ps_dS.tile([P, 2, 512], F32, tag="psS")
                        for tsub in range(6):
                            t = 10 + tsub
                            for nb in range(2):
                                nc.tensor.matmul(
                                    psS[:, nb], ktok[:, t, ts(dkc, P)],
                                    V[:, t, ts(nb, 512)],
                                    start=(tsub == 0), stop=(tsub == 5))
                        nc.vector.tensor_scalar_mul(
                            ccs2[:, dkc, 0:D],
                            psS.rearrange("p a b -> p (a b)"), fm1)
                    for kc in range(DK):
                        kred = p_kred.tile([P, 1], F32, tag="kred")
                        nc.vector.reduce_sum(kred, KT[:, kc, ds(10 * P, 768)],
                                             axis=AX)
                        nc.vector.tensor_scalar_mul(ccs2[:, kc, D:D + 1], kred, fm1)

                    for ch in range(NCH):
                        qs = ts(ch, C)
                        # --- scores + den ---
                        psD = ps_den.tile([1, C], F32, tag="psD")
                        ssbs = []
                        for cpi in range(2):
                            cp = 2 * ch + cpi
                            psc = ps_sc.tile([P, C], F32, tag="psc")
                            for kc in range(DK):
                                nc.tensor.matmul(psc, KT[:, kc, ts(cp, P)],
                                                 QT[:, kc, qs],
                                                 start=(kc == 0), stop=(kc == DK - 1))
                            ssb = p_ssb.tile([P, C], BF16, tag="ssb")
                            if cpi == 0:
                                nc.vector.tensor_tensor(ssb[:, 0:P], psc[:, 0:P],
                                                        tril, OP.mult)
                                nc.any.tensor_copy(ssb[:, P:C], psc[:, P:C])
                            else:
                                nc.vector.memset(ssb[:, 0:P], 0.0)
                                nc.vector.tensor_tensor(ssb[:, P:C], psc[:, P:C],
                                                        tril, OP.mult)
                            ssbs.append(ssb)
                            nc.tensor.matmul(psD, ones_b, ssb, start=(cpi == 0),
                                             stop=(ch == 0 and cpi == 1))
                        if ch > 0:
                            for kc in range(DK):
                                nc.tensor.matmul(psD, S[:, kc, D:D + 1],
                                                 QT[:, kc, qs],
                                                 start=False, stop=(kc == DK - 1))
                        dsc = p_kred.tile([1, C], F32, tag="dsc")
                        nc.vector.tensor_copy(dsc, psD)
                        nc.sync.dma_start(
                            den_nof[ds(ch * C, C)].rearrange("(a q) -> a q", a=1),
                            dsc)

                        # --- num: intra + cross(own prefix) ---
                        for pp in range(4):
                            psN = ps_num.tile([P, 2, 512], F32, tag="psN")
                            for k2 in range(2):
                                dvc = 2 * pp + k2
                                for cpi in range(2):
                                    nc.tensor.matmul(
                                        psN[:, k2, 0:C],
                                        V[:, 2 * ch + cpi, ts(dvc, P)], ssbs[cpi],
                                        start=(cpi == 0),
                                        stop=(ch == 0 and cpi == 1))
                                if ch > 0:
                                    for kc in range(DK):
                                        nc.tensor.matmul(
                                            psN[:, k2, 0:C],
                                            S[:, kc, ts(dvc, P)], QT[:, kc, qs],
                                            start=False, stop=(kc == DK - 1))
                                nc.any.tensor_copy(num[:, 2 * pp + k2, qs],
                                                   psN[:, k2, 0:C])

                        # --- dS accumulated into S (skip last chunk: the
                        # boundary state already went out via ccs2) ---
                        if ch < NCH - 1:
                            for dkc in range(DK):
                                psS = ps_dS.tile([P, 2, 512], F32, tag="psS")
                                for tsub in range(2):
                                    t = 2 * ch + tsub
                                    for nb in range(2):
                                        nc.tensor.matmul(
                                            psS[:, nb], ktok[:, t, ts(dkc, P)],
                                            V[:, t, ts(nb, 512)],
                                            start=(tsub == 0), stop=(tsub == 1))
                                nc.vector.tensor_tensor(
                                    S[:, dkc, 0:D],
                                    psS.rearrange("p a b -> p (a b)"),
                                    S[:, dkc, 0:D], OP.add)
                            for kc in range(DK):
                                kred = p_kred.tile([P, 1], F32, tag="kred")
                                nc.vector.reduce_sum(kred, KT[:, kc, qs], axis=AX)
                                nc.vector.tensor_tensor(S[:, kc, D:D + 1], kred,
                                                        S[:, kc, D:D + 1], OP.add)

                        if ch == NCH - 4:
                            # prefix5 is now in S: finish the gather input
                            # (ccs2 += S * (1-flag)) and launch the collective
                            nc.vector.scalar_tensor_tensor(
                                out=ccs2, in0=S, scalar=fm1, in1=ccs2,
                                op0=OP.mult, op1=OP.add)
                            nc.sync.dma_start(
                                cc_in.rearrange("(o p) m -> p o m", p=P), ccs2)
                            nc.gpsimd.collective_compute(
                                "AllGather", OP.bypass,
                                replica_groups=[[0, 1], [2, 3], [4, 5], [6, 7]],
                                ins=[cc_in.opt()], outs=[cc_out.opt()])

                p_v_cm.__exit__(None, None, None)

            # ======== post-pass (E terms) fused with out-projection ========
            with tc.tile_pool(name="p_E", bufs=1) as p_E, \
                 tc.tile_pool(name="p_wp", bufs=1) as p_wp, \
                 tc.tile_pool(name="p_df", bufs=3) as p_df, \
                 tc.tile_pool(name="p_osb", bufs=3) as p_osb, \
                 tc.tile_pool(name="ps_pn", bufs=1, space="PSUM") as ps_pn, \
                 tc.tile_pool(name="ps_pd", bufs=1, space="PSUM") as ps_pd, \
                 tc.tile_pool(name="ps_o", bufs=2, space="PSUM") as ps_o:
                wpt = p_wp.tile([P, DK, D], BF16)
                nc.sync.dma_start(wpt, wp.rearrange("(o p) m -> p o m", p=P))
                E = p_E.tile([P, DK, D + 1], BF16)   # partner state, unscaled
                nc.sync.dma_start(E, cc_out[0].rearrange("(o p) m -> p o m", p=P))
                if "dbg_e" in io:
                    nc.sync.dma_start(io["dbg_e"].rearrange("(o p) m -> p o m", p=P), E)

                for chp in range(NCH // 2):
                    qs2 = ts(chp, 512)
                    for pp in range(4):
                        psN2 = ps_pn.tile([P, 2, 512], F32, tag="psN2")
                        for k2 in range(2):
                            dvc = 2 * pp + k2
                            for kc in range(DK):
                                nc.tensor.matmul(
                                    psN2[:, k2], E[:, kc, ts(dvc, P)],
                                    QT[:, kc, qs2],
                                    start=(kc == 0), stop=(kc == DK - 1))
                            # num += E-term * flag (flag gates first-half cores)
                            nc.vector.scalar_tensor_tensor(
                                out=num[:, dvc, qs2], in0=psN2[:, k2],
                                scalar=flag_sb, in1=num[:, dvc, qs2],
                                op0=OP.mult, op1=OP.add)
                    psD2 = ps_pd.tile([1, 512], F32, tag="psD2")
                    for kc in range(DK):
                        nc.tensor.matmul(psD2, E[:, kc, D:D + 1], QT[:, kc, qs2],
                                         start=(kc == 0), stop=(kc == DK - 1))
                    dnl = p_df.tile([1, 512], F32, tag="dnl")
                    nc.sync.dma_start(dnl, den_nof[ds(chp * 512, 512)].rearrange(
                        "(a q) -> a q", a=1))
                    dfin = p_df.tile([1, 512], F32, tag="dfin")
                    nc.vector.scalar_tensor_tensor(
                        out=dfin, in0=psD2, scalar=flag_sb[0:1, 0:1],
                        in1=dnl,
                        op0=OP.mult, op1=OP.add)
                    nc.vector.tensor_scalar_add(dfin, dfin, DEN_EPS)
                    nc.vector.reciprocal(dfin, dfin)
                    nc.sync.dma_start(
                        den_dram[ds(chp * 512, 512)].rearrange("(a q) -> a q", a=1),
                        dfin)

                    # --- out-projection for these 512 tokens ---
                    for tsub in range(4):
                        t = 4 * chp + tsub
                        pso = ps_o.tile([P, 2, 512], F32, tag="pso")
                        for dvc in range(DK):
                            for nb in range(2):
                                nc.tensor.matmul(
                                    pso[:, nb], num[:, dvc, ts(t, P)],
                                    wpt[:, dvc, ts(nb, 512)],
                                    start=(dvc == 0), stop=(dvc == DK - 1))
                        rden = p_osb.tile([P, 1], F32, tag="rden")
                        nc.sync.dma_start(rden, den_dram[ts(t, P)].rearrange(
                            "(p o) -> p o", o=1))
                        osb = p_osb.tile([P, D], F32, tag="osb")
                        pso_flat = pso.rearrange("p a b -> p (a b)")
                        if use_bias["bp"]:
                            nc.vector.scalar_tensor_tensor(
                                out=osb, in0=pso_flat, scalar=rden,
                                in1=bias_sb["bp"], op0=OP.mult, op1=OP.add)
                        else:
                            nc.vector.tensor_scalar_mul(osb, pso_flat, rden)
                        nc.sync.dma_start(out[ts(t, P), :], osb)

            if "dbg_num" in io:
                for j in range(DK):
                    nc.sync.dma_start(io["dbg_num"][ts(j, P), :], num[:, j, :])
                nc.sync.dma_start(io["dbg_den"].rearrange("(a q) -> a q", a=1),
                                  den_dram.rearrange("(a q) -> a q", a=1))


def build(TL, use_bias, debug=False):
    nc = bacc.Bacc("TRN2", target_bir_lowering=False, debug=False, num_devices=8)
    io = {}
    io["x"] = nc.dram_tensor("x", [TL, D], F32, kind="ExternalInput").ap()
    for nm in ("wg", "wk", "wq", "wv", "wp"):
        io[nm] = nc.dram_tensor(nm, [D, D], BF16, kind="ExternalInput").ap()
    io["flag"] = nc.dram_tensor("flag", [1, 1], F32, kind="ExternalInput").ap()
    for nm in ("bq", "bk", "bg", "bv", "bp"):
        if use_bias[nm]:
            io[nm] = nc.dram_tensor(nm, [D], F32, kind="ExternalInput").ap()
    io["out"] = nc.dram_tensor("out", [TL, D], F32, kind="ExternalOutput").ap()
    if debug:
        io["dbg_kt"] = nc.dram_tensor("dbg_kt", [D, TL], BF16, kind="ExternalOutput").ap()
        io["dbg_qt"] = nc.dram_tensor("dbg_qt", [D, TL], BF16, kind="ExternalOutput").ap()
        io["dbg_e"] = nc.dram_tensor("dbg_e", [D, D + 1], BF16, kind="ExternalOutput").ap()
        io["dbg_num"] = nc.dram_tensor("dbg_num", [D, TL], BF16, kind="ExternalOutput").ap()
        io["dbg_den"] = nc.dram_tensor("dbg_den", [TL], F32, kind="ExternalOutput").ap()
    with tile.TileContext(nc) as tc:
        _emit(tc, io, TL, use_bias)
    nc.compile()
    return nc


_CACHE = {}


def _get_nc(TL, use_bias, debug=False):
    key = (TL, tuple(sorted(use_bias.items())), debug)
    if key not in _CACHE:
        _CACHE[key] = build(TL, use_bias, debug=debug)
    return _CACHE[key]


def kernel(x, w_qkv, b_qkv, w_gate, b_gate, w_proj, b_proj, ln_g, ln_b,
           run_kwargs=None, debug=False, **kw):
    run_kwargs = run_kwargs or {}
    x = np.asarray(x, np.float32)
    w_qkv = np.asarray(w_qkv, np.float32)
    b_qkv = np.asarray(b_qkv, np.float32)
    w_gate = np.asarray(w_gate, np.float32)
    b_gate = np.asarray(b_gate, np.float32)
    w_proj = np.asarray(w_proj, np.float32)
    b_proj = np.asarray(b_proj, np.float32)
    ln_g = np.asarray(ln_g, np.float32)
    ln_b = np.asarray(ln_b, np.float32)

    TL = T_FULL // 2
    # fold LayerNorm affine into the first-layer weights/biases
    g = ln_g[:, None]
    weights = {
        "wq": np.ascontiguousarray((g * w_qkv[:, :D]).astype(BF16_NP)),
        "wk": np.ascontiguousarray((g * w_qkv[:, D:2 * D]).astype(BF16_NP)),
        "wv": np.ascontiguousarray((g * w_qkv[:, 2 * D:]).astype(BF16_NP)),
        "wg": np.ascontiguousarray((g * w_gate).astype(BF16_NP)),
        "wp": np.ascontiguousarray(w_proj.astype(BF16_NP)),
    }
    biases = {
        "bq": ln_b @ w_qkv[:, :D] + b_qkv[:D],
        "bk": ln_b @ w_qkv[:, D:2 * D] + b_qkv[D:2 * D],
        "bv": ln_b @ w_qkv[:, 2 * D:] + b_qkv[2 * D:],
        "bg": ln_b @ w_gate + b_gate,
        "bp": b_proj,
    }
    use_bias = {nm: bool(np.any(v)) for nm, v in biases.items()}
    nc = _get_nc(TL, use_bias, debug=debug)

    in_maps = []
    for c in range(8):
        b, h = c // 2, c % 2
        m = {
            "x": np.ascontiguousarray(x[b, h * TL:(h + 1) * TL]),
            "flag": np.array([[float(h)]], np.float32),
            **weights,
        }
        for nm in ("bq", "bk", "bg", "bv", "bp"):
            if use_bias[nm]:
                m[nm] = np.ascontiguousarray(biases[nm].astype(np.float32))
        in_maps.append(m)

    res = run_bass_kernel_spmd(nc, in_maps, core_ids=list(range(8)), **run_kwargs)
    out = np.empty((B_FULL, T_FULL, D), np.float32)
    for c in range(8):
        b, h = c // 2, c % 2
        out[b, h * TL:(h + 1) * TL] = res.results[c]["out"]
    if run_kwargs or debug:
        return out, res
    return out
